# revision 14
# baseline (speedup 1.0000x reference)
"""Trainium2 Bass kernel for nn_MultiHeadAttention_91027536871977.

Cosine-similarity multi-head self-attention:
  x      = einsum("bsd,hdf->bhsf", sin, Wx) + bx          [B,H,S,F]
  scores = (x @ x^T) / (|x| |x|^T)                        [B,H,S,S]
  p      = softmax(scores, -1)
  out    = concat_heads(p @ x) @ Wp + bp                  [B,S,D]

Sharding: pure data-parallel over batch (B=8 -> 8 cores, one batch each,
all 16 heads + the output projection local to the core; no collectives).

v2 schedule (vs the 299us baseline):
  - x stored [t_p, tile, h, 65] with a ones column per head; the out^T
    matmuls use lhsT = [x_h | 1] (M=65) so PSUM row 64 accumulates
    rs = sum_t E[t,s] for free -> no ACT accum_out (saves ~40us of
    ACTIVATION_READ_ACCUMULATOR) and no selector-matmul rs chain.
  - 1/rs: DVE reciprocal of the PSUM rs row (bf16), GpSimd
    partition_broadcast to all 128 partitions, then mixed-partition-base
    DVE muls scale both heads' out^T halves (head1 writes parts 64-127
    directly from the base-0 PSUM tile).
  - |x|^2 (square + reduce) moved to GpSimd; DVE keeps only PSUM-touching
    work.
  - Y projection split: pairs 0-3 projected mid-loop into yA (f32 SBUF),
    pairs 4-7 + final add in the tail -> shorter ACT-idle tail.
  - Loop orders chosen so consecutive matmuls reuse the stationary
    operand where possible (fewer LDWEIGHTS stalls), and the PE stream is
    kept dense to hold the HAM clock gate at 2.4 GHz.
"""

import numpy as np
import ml_dtypes

import concourse.bass as bass
import concourse.bacc as bacc
import concourse.mybir as mybir
import concourse.tile as tile
from concourse.bass_utils import run_bass_kernel_spmd

B, S, D, H, F = 8, 1024, 1024, 16, 64
P = 128
NP = H // 2  # head pairs
KO = D // P  # k subtiles
NT = S // P  # s tiles
BF16 = mybir.dt.bfloat16
F32 = mybir.dt.float32
HALF = S // 2
FP = F + 1  # per-head x columns incl. the ones column


def build_program() -> bass.Bass:
    nc = bacc.Bacc("TRN2", target_bir_lowering=False, debug=False)

    d_sint = nc.dram_tensor("sint", [D, S], BF16, kind="ExternalInput")
    d_wx = nc.dram_tensor("wx", [D, H * F], BF16, kind="ExternalInput")
    d_wp = nc.dram_tensor("wp", [H * F, D], BF16, kind="ExternalInput")
    d_bxf = nc.dram_tensor("bxf", [1, H * F], F32, kind="ExternalInput")
    d_bp = nc.dram_tensor("bp", [1, D], F32, kind="ExternalInput")
    d_sel8 = nc.dram_tensor("sel8", [2 * NT, NT, P], BF16, kind="ExternalInput")
    d_ident = nc.dram_tensor("ident", [P, P], BF16, kind="ExternalInput")
    d_y = nc.dram_tensor("y", [S, D], F32, kind="ExternalOutput")

    with tile.TileContext(nc) as tc:
        _body(tc, d_sint, d_wx, d_wp, d_bxf, d_bp, d_sel8, d_ident, d_y)
    nc.compile()
    return nc


def _bcast_rows(dram_ap, parts=P):
    """DMA access pattern replicating a [1, N] DRAM row across `parts` partitions."""
    return bass.AP(
        tensor=dram_ap.tensor,
        offset=dram_ap.offset,
        ap=[[0, parts]] + list(dram_ap.ap[1:]),
    )


def _body(tc, d_sint, d_wx, d_wp, d_bxf, d_bp, d_sel8, d_ident, d_y):
    nc = tc.nc
    from contextlib import ExitStack

    with ExitStack() as ctx:
        singles = ctx.enter_context(tc.tile_pool(name="singles", bufs=1))
        wtiles = ctx.enter_context(tc.tile_pool(name="wtiles", bufs=2))
        e_pool = ctx.enter_context(tc.tile_pool(name="epool", bufs=4))
        b_pool = ctx.enter_context(tc.tile_pool(name="bpool", bufs=2))
        rc_pool = ctx.enter_context(tc.tile_pool(name="rcpool", bufs=2))
        y_pool = ctx.enter_context(tc.tile_pool(name="ypool", bufs=2))
        bc_pool = ctx.enter_context(tc.tile_pool(name="bcpool", bufs=1))

        # PSUM is bank-granular (8 x 2KB): big 2x2 banks + prep 2x1 bank
        # (bufs=1 per tag) + ot 2x1 bank = 8 banks
        ps_big = ctx.enter_context(tc.tile_pool(name="ps_big", bufs=2, space="PSUM"))
        ps_prep = ctx.enter_context(tc.tile_pool(name="ps_prep", bufs=1, space="PSUM"))
        ps_ot = ctx.enter_context(tc.tile_pool(name="ps_ot", bufs=2, space="PSUM"))

        # ---- load everything to SBUF ----
        # sint/wx are dead after the X projection; yA later rotates into
        # sint's buffer via the shared 2-buf tag.
        sint_sb = wtiles.tile([P, KO, S], BF16, tag="w", name="sint_sb")
        wx_sb = wtiles.tile([P, KO, H * F], BF16, tag="w", name="wx_sb")
        sint_r = d_sint.rearrange("(ko p) s -> p ko s", p=P)
        wx_r = d_wx.rearrange("(ko p) n -> p ko n", p=P)
        for ko in range(KO):
            nc.sync.dma_start(wx_sb[:, ko, :], wx_r[:, ko, :])
            nc.sync.dma_start(sint_sb[:, ko, :], sint_r[:, ko, :])
        wp_sb = singles.tile([P, KO, D], BF16)
        nc.sync.dma_start(wp_sb, d_wp.rearrange("(ko p) n -> p ko n", p=P))
        bxf_sb = bc_pool.tile([P, H * F], F32, tag="bc", name="bxf_sb")
        nc.gpsimd.dma_start(bxf_sb, _bcast_rows(d_bxf[:, :]))
        bp_sb = bc_pool.tile([P, D], F32, tag="bc", name="bp_sb")
        nc.gpsimd.dma_start(bp_sb, _bcast_rows(d_bp[:, :]))
        sel8_sb = singles.tile([2 * NT, NT, P], BF16)
        nc.sync.dma_start(sel8_sb, d_sel8[:, :, :])
        ident_sb = singles.tile([P, P], BF16)
        nc.sync.dma_start(ident_sb, d_ident[:, :])

        # persistent intermediates
        x_sb = singles.tile([P, NT, H, FP], BF16)   # x + ones col, [t_p, tile, h, f|1]
        xtn_sb = singles.tile([P, NP, S], BF16)     # normalized x^T [f2, pair, t]
        outt_sb = singles.tile([P, NP, S], BF16)    # attention out^T [f2, pair, s]
        n2s_sb = singles.tile([P, P], F32)          # |x|^2 [s_p, col h*8+i]
        nrcp_sb = singles.tile([P, P], F32)         # 1/|x|^2
        invs_sb = singles.tile([P, P], BF16)        # 1/|x| [s_p, col h*8+i]
        ya_ref = {}  # Y partial (pairs 0-3) + bias; allocated lazily

        # ones columns of x (written once; bias-add below fills cols 0:64)
        nc.vector.memset(x_sb[:, :, :, F:FP], 1.0)

        # ---- X = sin @ Wx + bx;  |x|^2 on GpSimd ----
        for i in range(NT):
            x_ps = ps_big.tile([P, S], F32, tag="big", name=f"x_{i}")
            for ko in range(KO):
                for hlf in range(2):
                    nc.tensor.matmul(
                        x_ps[:, hlf * HALF:(hlf + 1) * HALF],
                        lhsT=sint_sb[:, ko, i * P:(i + 1) * P],
                        rhs=wx_sb[:, ko, hlf * HALF:(hlf + 1) * HALF],
                        start=(ko == 0), stop=(ko == KO - 1),
                        skip_group_check=True,
                    )
            nc.vector.tensor_add(
                x_sb[:, i, :, 0:F],
                x_ps.rearrange("p (h f) -> p h f", f=F),
                bxf_sb.rearrange("p (h f) -> p h f", f=F),
            )
            xsq = b_pool.tile([P, H * F], BF16, tag="xsq", name=f"xsq_{i}")
            nc.gpsimd.tensor_mul(
                xsq.rearrange("p (h f) -> p h f", f=F),
                x_sb[:, i, :, 0:F], x_sb[:, i, :, 0:F],
            )
            nc.vector.reduce_sum(
                n2s_sb.rearrange("p (hh ii) -> p hh ii", ii=NT)[:, :, i],
                xsq.rearrange("p (hh f) -> p hh f", f=F),
                axis=mybir.AxisListType.X,
            )
        nc.vector.reciprocal(nrcp_sb, n2s_sb)
        nc.scalar.sqrt(invs_sb, nrcp_sb)

        def prep(q):
            """xtn for pair q: transpose x columns, scale by 1/|x| bcast."""
            invq_ps = ps_prep.tile([P, P], BF16, tag="xtt", name=f"invq_{q}")
            nc.tensor.transpose(
                invq_ps[0:2 * NT, :], invs_sb[:, q * 2 * NT:(q + 1) * 2 * NT],
                ident_sb)
            invq_sb = b_pool.tile([2 * NT, P], BF16, tag="rcpq", name=f"invqs_{q}")
            nc.vector.tensor_copy(invq_sb, invq_ps[0:2 * NT, :])
            nrm_sb = b_pool.tile([P, NT, P], BF16, tag="nrm", name=f"nrm_{q}")
            for j in range(NT):
                nrm_ps = ps_prep.tile([P, P], F32, tag="nrmp", name=f"nrmp_{q}_{j}")
                nc.tensor.matmul(
                    nrm_ps, lhsT=sel8_sb[:, j, :], rhs=invq_sb,
                    start=True, stop=True,
                )
                nc.vector.tensor_copy(nrm_sb[:, j, :], nrm_ps)
                xtt_ps = ps_prep.tile([P, P], BF16, tag="xtt", name=f"xtt_{q}_{j}")
                nc.tensor.transpose(
                    xtt_ps[0:F, :], x_sb[:, j, 2 * q, 0:F], ident_sb)
                nc.tensor.transpose(
                    xtt_ps[F:2 * F, :], x_sb[:, j, 2 * q + 1, 0:F], ident_sb)
                nc.vector.tensor_mul(
                    xtn_sb[:, q, j * P:(j + 1) * P], xtt_ps, nrm_sb[:, j, :])

        e_store = {}

        def gram_tile(q, i, hh):
            """Gram + exp for head hh of pair q at s-tile i."""
            g_ps = ps_big.tile([P, S], F32, tag="big", name=f"g_{q}_{hh}_{i}")
            frows = slice(hh * F, (hh + 1) * F)
            for hlf in range(2):
                nc.tensor.matmul(
                    g_ps[:, hlf * HALF:(hlf + 1) * HALF],
                    lhsT=xtn_sb[frows, q, i * P:(i + 1) * P],
                    rhs=xtn_sb[frows, q, hlf * HALF:(hlf + 1) * HALF],
                    start=True, stop=True,
                    skip_group_check=True,
                )
            nc.scalar.activation(
                e_store[q][hh][:, i, :], g_ps,
                mybir.ActivationFunctionType.Exp,
            )

        def alloc_e(q):
            e_store[q] = [
                e_pool.tile([P, NT, S], BF16, tag="e", name=f"e_{q}_{hh}")
                for hh in range(2)]

        def ot_chain(q, hh, hlf):
            """out^T accumulation for (pair q, head hh, s-half hlf) with the
            ones-column rs trick; returns nothing (writes outt_sb)."""
            ot = ps_ot.tile([FP, HALF], F32, tag="ot", name=f"ot_{q}_{hh}_{hlf}")
            for j in range(NT):
                nc.tensor.matmul(
                    ot,
                    lhsT=x_sb[:, j, 2 * q + hh, :],
                    rhs=e_store[q][hh][:, j, hlf * HALF:(hlf + 1) * HALF],
                    start=(j == 0), stop=(j == NT - 1),
                    skip_group_check=True,
                )
            rcp = rc_pool.tile([P, HALF], BF16, tag="rcp", name=f"rcp_{q}_{hh}_{hlf}")
            with nc.allow_low_precision(reason="1/rs in bf16 as in baseline"):
                # partition shift 64 -> 0: partition_broadcast reads its
                # source tile's partition 0
                nc.vector.reciprocal(rcp[0:1, :], ot[F:FP, :])
            brc = rc_pool.tile([P, HALF], BF16, tag="brc", name=f"brc_{q}_{hh}_{hlf}")
            nc.gpsimd.partition_broadcast(brc, rcp[0:1, :])
            cols = slice(hlf * HALF, (hlf + 1) * HALF)
            if hh == 0:
                nc.vector.tensor_mul(
                    outt_sb[0:F, q, cols], ot[0:F, :], brc[0:F, :])
            else:
                nc.vector.tensor_mul(
                    outt_sb[F:2 * F, q, cols], ot[0:F, :], brc[F:2 * F, :])

        def y_proj(i, q0, q1, y_ps):
            """Y contribution of pairs [q0, q1) for s-tile i into y_ps."""
            for q in range(q0, q1):
                for hlf in range(2):
                    nc.tensor.matmul(
                        y_ps[:, hlf * HALF:(hlf + 1) * HALF],
                        lhsT=outt_sb[:, q, i * P:(i + 1) * P],
                        rhs=wp_sb[:, q, hlf * HALF:(hlf + 1) * HALF],
                        start=(q == q0), stop=(q == q1 - 1),
                        skip_group_check=True,
                    )

        # ---- prep all pairs, then software-pipelined attention ----
        for q in range(NP):
            prep(q)
        alloc_e(0)

        # gram(0) emitted alone; exps start as soon as tiles land
        for i in range(NT):
            for hh in range(2):
                gram_tile(0, i, hh)

        for q in range(NP):
            nxt = q + 1 if q + 1 < NP else None
            if nxt is not None:
                alloc_e(nxt)
            # interleave: gram(nxt) paced by ACT; out^T(q) + chains fill PE;
            # Y partial for pairs 0-3 emitted in windows q>=5
            chains = [(hh, hlf) for hh in range(2) for hlf in range(2)]
            for step in range(NT * 2):  # 16 gram steps per pair
                i, hh = step // 2, step % 2
                if nxt is not None:
                    gram_tile(nxt, i, hh)
                if step % 4 == 3:
                    c_hh, c_hlf = chains[step // 4]
                    ot_chain(q, c_hh, c_hlf)
            if q >= 5:
                # Y partial (pairs 0-3) for tiles assigned to this window
                if "ya" not in ya_ref:
                    ya_ref["ya"] = wtiles.tile([P, KO, S], BF16, tag="w",
                                               name="ya_sb")
                ya_sb = ya_ref["ya"]
                lo = (q - 5) * 3
                hi = min(lo + 3, NT)
                for i in range(lo, hi):
                    y_ps = ps_big.tile([P, D], F32, tag="big", name=f"ya_{i}")
                    y_proj(i, 0, NP // 2, y_ps)
                    nc.vector.tensor_add(ya_sb[:, i, :], y_ps, bp_sb)
            if q in e_store and q < NP - 1:
                del e_store[q]

        # tail: out^T(7) chains then Y_B + final add + DMA out
        for hh in range(2):
            for hlf in range(2):
                ot_chain(NP - 1, hh, hlf)
        del e_store[NP - 1]

        ya_sb = ya_ref["ya"]
        for i in range(NT):
            y_ps = ps_big.tile([P, D], F32, tag="big", name=f"yb_{i}")
            y_proj(i, NP // 2, NP, y_ps)
            y_sb = y_pool.tile([P, D], F32, tag="y", name=f"ys_{i}")
            nc.vector.tensor_add(y_sb, y_ps, ya_sb[:, i, :])
            nc.sync.dma_start(d_y[i * P:(i + 1) * P, :], y_sb)


_CACHE: dict = {}


def _get_program() -> bass.Bass:
    if "nc" not in _CACHE:
        _CACHE["nc"] = build_program()
    return _CACHE["nc"]


def _prep_inputs(sin, Wx, bx, Wp, bp):
    """Host-side sharding + layout prep. Returns per-core input maps."""
    bf16 = ml_dtypes.bfloat16
    wx_flat = np.ascontiguousarray(
        np.transpose(np.asarray(Wx, np.float32), (1, 0, 2)).reshape(D, H * F)
    ).astype(bf16)
    wp_b = np.ascontiguousarray(np.asarray(Wp, np.float32)).astype(bf16)
    bx32 = np.asarray(bx, np.float32)
    bxf = np.ascontiguousarray(bx32.reshape(1, H * F))
    bp32 = np.ascontiguousarray(np.asarray(bp, np.float32).reshape(1, D))
    # sel8[r][j][p] = 1 iff r == (p//64)*8 + j  (broadcasts invq rows j and
    # 8+j of a pair's [16,128] 1/|x| tile to partitions 0-63 / 64-127)
    sel8 = np.zeros((2 * NT, NT, P), np.float32)
    for j in range(NT):
        sel8[j, j, :F] = 1.0
        sel8[NT + j, j, F:] = 1.0
    sel8 = sel8.astype(bf16)
    ident = np.eye(P, dtype=np.float32).astype(bf16)

    sin32 = np.asarray(sin, np.float32)
    in_maps = []
    for b in range(B):
        sint = np.ascontiguousarray(sin32[b].T).astype(bf16)
        in_maps.append({
            "sint": sint, "wx": wx_flat, "wp": wp_b, "bxf": bxf,
            "bp": bp32, "sel8": sel8, "ident": ident,
        })
    return in_maps


def kernel(sin, mask, Wx, bx, Wp, bp, _run_kwargs=None):
    nc = _get_program()
    in_maps = _prep_inputs(sin, Wx, bx, Wp, bp)
    res = run_bass_kernel_spmd(nc, in_maps, core_ids=list(range(B)),
                               **(_run_kwargs or {}))
    out = np.stack([np.asarray(res.results[b]["y"], np.float32) for b in range(B)])
    if _run_kwargs:
        _CACHE["last_results"] = res
    return out


# revision 15
# speedup vs baseline: 1.2497x; 1.2497x over previous
"""Trainium2 Bass kernel for nn_MultiHeadAttention_91027536871977.

Cosine-similarity multi-head self-attention:
  x      = einsum("bsd,hdf->bhsf", sin, Wx) + bx          [B,H,S,F]
  scores = (x @ x^T) / (|x| |x|^T)                        [B,H,S,S]
  p      = softmax(scores, -1)
  out    = concat_heads(p @ x) @ Wp + bp                  [B,S,D]

Sharding: pure data-parallel over batch (B=8 -> 8 cores, one batch each,
all 16 heads + the output projection local to the core; no collectives).

v2 schedule (vs the 299us baseline):
  - x stored [t_p, tile, h, 65] with a ones column per head; the out^T
    matmuls use lhsT = [x_h | 1] (M=65) so PSUM row 64 accumulates
    rs = sum_t E[t,s] for free -> no ACT accum_out (saves ~40us of
    ACTIVATION_READ_ACCUMULATOR) and no selector-matmul rs chain.
  - 1/rs: DVE reciprocal of the PSUM rs row (bf16), GpSimd
    partition_broadcast to all 128 partitions, then mixed-partition-base
    DVE muls scale both heads' out^T halves (head1 writes parts 64-127
    directly from the base-0 PSUM tile).
  - |x|^2 (square + reduce) moved to GpSimd; DVE keeps only PSUM-touching
    work.
  - Y projection split: pairs 0-3 projected mid-loop into yA (f32 SBUF),
    pairs 4-7 + final add in the tail -> shorter ACT-idle tail.
  - Loop orders chosen so consecutive matmuls reuse the stationary
    operand where possible (fewer LDWEIGHTS stalls), and the PE stream is
    kept dense to hold the HAM clock gate at 2.4 GHz.
"""

import numpy as np
import ml_dtypes

import concourse.bass as bass
import concourse.bacc as bacc
import concourse.mybir as mybir
import concourse.tile as tile
from concourse.bass_utils import run_bass_kernel_spmd

B, S, D, H, F = 8, 1024, 1024, 16, 64
P = 128
NP = H // 2  # head pairs
KO = D // P  # k subtiles
NT = S // P  # s tiles
BF16 = mybir.dt.bfloat16
F32 = mybir.dt.float32
HALF = S // 2
FP = F + 1  # per-head x columns incl. the ones column


def build_program() -> bass.Bass:
    nc = bacc.Bacc("TRN2", target_bir_lowering=False, debug=False)

    d_sint = nc.dram_tensor("sint", [D, S], BF16, kind="ExternalInput")
    d_wx = nc.dram_tensor("wx", [D, H * F], BF16, kind="ExternalInput")
    d_wp = nc.dram_tensor("wp", [H * F, D], BF16, kind="ExternalInput")
    d_bxf = nc.dram_tensor("bxf", [1, H * F], F32, kind="ExternalInput")
    d_bp = nc.dram_tensor("bp", [1, D], F32, kind="ExternalInput")
    d_sel8 = nc.dram_tensor("sel8", [2 * NT, NT, P], BF16, kind="ExternalInput")
    d_ident = nc.dram_tensor("ident", [P, P], BF16, kind="ExternalInput")
    d_y = nc.dram_tensor("y", [S, D], F32, kind="ExternalOutput")

    with tile.TileContext(nc) as tc:
        _body(tc, d_sint, d_wx, d_wp, d_bxf, d_bp, d_sel8, d_ident, d_y)
    nc.compile()
    return nc


def _bcast_rows(dram_ap, parts=P):
    """DMA access pattern replicating a [1, N] DRAM row across `parts` partitions."""
    return bass.AP(
        tensor=dram_ap.tensor,
        offset=dram_ap.offset,
        ap=[[0, parts]] + list(dram_ap.ap[1:]),
    )


def _body(tc, d_sint, d_wx, d_wp, d_bxf, d_bp, d_sel8, d_ident, d_y):
    nc = tc.nc
    from contextlib import ExitStack

    with ExitStack() as ctx:
        singles = ctx.enter_context(tc.tile_pool(name="singles", bufs=1))
        wtiles = ctx.enter_context(tc.tile_pool(name="wtiles", bufs=2))
        e_pool = ctx.enter_context(tc.tile_pool(name="epool", bufs=4))
        b_pool = ctx.enter_context(tc.tile_pool(name="bpool", bufs=2))
        rc_pool = ctx.enter_context(tc.tile_pool(name="rcpool", bufs=2))
        y_pool = ctx.enter_context(tc.tile_pool(name="ypool", bufs=2))
        bc_pool = ctx.enter_context(tc.tile_pool(name="bcpool", bufs=1))

        # PSUM is bank-granular (8 x 2KB): big 2x2 banks + prep 2x1 bank
        # (bufs=1 per tag) + ot 2x1 bank = 8 banks
        ps_big = ctx.enter_context(tc.tile_pool(name="ps_big", bufs=2, space="PSUM"))
        ps_prep = ctx.enter_context(tc.tile_pool(name="ps_prep", bufs=1, space="PSUM"))
        ps_ot = ctx.enter_context(tc.tile_pool(name="ps_ot", bufs=2, space="PSUM"))

        # ---- load everything to SBUF ----
        # sint/wx are dead after the X projection; yA later rotates into
        # sint's buffer via the shared 2-buf tag.
        sint_sb = wtiles.tile([P, KO, S], BF16, tag="w", name="sint_sb")
        wx_sb = wtiles.tile([P, KO, H * F], BF16, tag="w", name="wx_sb")
        sint_r = d_sint.rearrange("(ko p) s -> p ko s", p=P)
        wx_r = d_wx.rearrange("(ko p) n -> p ko n", p=P)
        for ko in range(KO):
            nc.sync.dma_start(wx_sb[:, ko, :], wx_r[:, ko, :])
            nc.sync.dma_start(sint_sb[:, ko, :], sint_r[:, ko, :])
        wp_sb = singles.tile([P, KO, D], BF16)
        nc.sync.dma_start(wp_sb, d_wp.rearrange("(ko p) n -> p ko n", p=P))
        bxf_sb = bc_pool.tile([P, H * F], F32, tag="bc", name="bxf_sb")
        nc.gpsimd.dma_start(bxf_sb, _bcast_rows(d_bxf[:, :]))
        bp_sb = bc_pool.tile([P, D], F32, tag="bc", name="bp_sb")
        nc.gpsimd.dma_start(bp_sb, _bcast_rows(d_bp[:, :]))
        sel8_sb = singles.tile([2 * NT, NT, P], BF16)
        nc.sync.dma_start(sel8_sb, d_sel8[:, :, :])
        ident_sb = singles.tile([P, P], BF16)
        nc.sync.dma_start(ident_sb, d_ident[:, :])

        # persistent intermediates
        x_sb = singles.tile([P, NT, H, FP], BF16)   # x + ones col, [t_p, tile, h, f|1]
        xtn_sb = singles.tile([P, NP, S], BF16)     # normalized x^T [f2, pair, t]
        outt_sb = singles.tile([P, NP, S], BF16)    # attention out^T [f2, pair, s]
        n2s_sb = singles.tile([P, P], F32)          # |x|^2 [s_p, col h*8+i]
        nrcp_sb = singles.tile([P, P], F32)         # 1/|x|^2
        invs_sb = singles.tile([P, P], BF16)        # 1/|x| [s_p, col h*8+i]
        ya_ref = {}  # Y partial (pairs 0-3) + bias; allocated lazily

        # ones columns of x (written once; bias-add below fills cols 0:64)
        nc.vector.memset(x_sb[:, :, :, F:FP], 1.0)

        # ---- X = sin @ Wx + bx;  |x|^2 on GpSimd ----
        for i in range(NT):
            x_ps = ps_big.tile([P, S], F32, tag="big", name=f"x_{i}")
            for ko in range(KO):
                for hlf in range(2):
                    nc.tensor.matmul(
                        x_ps[:, hlf * HALF:(hlf + 1) * HALF],
                        lhsT=sint_sb[:, ko, i * P:(i + 1) * P],
                        rhs=wx_sb[:, ko, hlf * HALF:(hlf + 1) * HALF],
                        start=(ko == 0), stop=(ko == KO - 1),
                        skip_group_check=True,
                    )
            nc.vector.tensor_add(
                x_sb[:, i, :, 0:F],
                x_ps.rearrange("p (h f) -> p h f", f=F),
                bxf_sb.rearrange("p (h f) -> p h f", f=F),
            )
            xsq = b_pool.tile([P, H * F], BF16, tag="xsq", name=f"xsq_{i}")
            nc.gpsimd.tensor_mul(
                xsq.rearrange("p (h f) -> p h f", f=F),
                x_sb[:, i, :, 0:F], x_sb[:, i, :, 0:F],
            )
            nc.vector.reduce_sum(
                n2s_sb.rearrange("p (hh ii) -> p hh ii", ii=NT)[:, :, i],
                xsq.rearrange("p (hh f) -> p hh f", f=F),
                axis=mybir.AxisListType.X,
            )
        nc.vector.reciprocal(nrcp_sb, n2s_sb)
        nc.scalar.sqrt(invs_sb, nrcp_sb)

        def prep(q):
            """xtn for pair q: transpose x columns, scale by 1/|x| bcast."""
            invq_ps = ps_prep.tile([P, P], BF16, tag="xtt", name=f"invq_{q}")
            nc.tensor.transpose(
                invq_ps[0:2 * NT, :], invs_sb[:, q * 2 * NT:(q + 1) * 2 * NT],
                ident_sb)
            invq_sb = b_pool.tile([2 * NT, P], BF16, tag="rcpq", name=f"invqs_{q}")
            nc.vector.tensor_copy(invq_sb, invq_ps[0:2 * NT, :])
            nrm_sb = b_pool.tile([P, NT, P], BF16, tag="nrm", name=f"nrm_{q}")
            for j in range(NT):
                nrm_ps = ps_prep.tile([P, P], F32, tag="nrmp", name=f"nrmp_{q}_{j}")
                nc.tensor.matmul(
                    nrm_ps, lhsT=sel8_sb[:, j, :], rhs=invq_sb,
                    start=True, stop=True,
                )
                nc.vector.tensor_copy(nrm_sb[:, j, :], nrm_ps)
                xtt_ps = ps_prep.tile([P, P], BF16, tag="xtt", name=f"xtt_{q}_{j}")
                nc.tensor.transpose(
                    xtt_ps[0:F, :], x_sb[:, j, 2 * q, 0:F], ident_sb)
                nc.tensor.transpose(
                    xtt_ps[F:2 * F, :], x_sb[:, j, 2 * q + 1, 0:F], ident_sb)
                nc.vector.tensor_mul(
                    xtn_sb[:, q, j * P:(j + 1) * P], xtt_ps, nrm_sb[:, j, :])

        e_store = {}

        def gram_tile(q, i, hh):
            """Gram + exp for head hh of pair q at s-tile i."""
            g_ps = ps_big.tile([P, S], F32, tag="big", name=f"g_{q}_{hh}_{i}")
            frows = slice(hh * F, (hh + 1) * F)
            for hlf in range(2):
                nc.tensor.matmul(
                    g_ps[:, hlf * HALF:(hlf + 1) * HALF],
                    lhsT=xtn_sb[frows, q, i * P:(i + 1) * P],
                    rhs=xtn_sb[frows, q, hlf * HALF:(hlf + 1) * HALF],
                    start=True, stop=True,
                    skip_group_check=True,
                )
            nc.scalar.activation(
                e_store[q][hh][:, i, :], g_ps,
                mybir.ActivationFunctionType.Exp,
            )

        def alloc_e(q):
            e_store[q] = [
                e_pool.tile([P, NT, S], BF16, tag="e", name=f"e_{q}_{hh}")
                for hh in range(2)]

        def ot_chain(q, hh, hlf):
            """out^T accumulation for (pair q, head hh, s-half hlf) with the
            ones-column rs trick; returns nothing (writes outt_sb)."""
            ot = ps_ot.tile([FP, HALF], F32, tag="ot", name=f"ot_{q}_{hh}_{hlf}")
            for j in range(NT):
                nc.tensor.matmul(
                    ot,
                    lhsT=x_sb[:, j, 2 * q + hh, :],
                    rhs=e_store[q][hh][:, j, hlf * HALF:(hlf + 1) * HALF],
                    start=(j == 0), stop=(j == NT - 1),
                    skip_group_check=True,
                )
            rcp = rc_pool.tile([P, HALF], F32, tag="rcp", name=f"rcp_{q}_{hh}_{hlf}")
            # partition shift 64 -> 0: partition_broadcast reads its source
            # tile's partition 0.  approx_fast: 18 bits, single DVE pass.
            nc.vector.reciprocal_approx_fast(rcp[0:1, :], ot[F:FP, :])
            brc = rc_pool.tile([P, HALF], F32, tag="brc", name=f"brc_{q}_{hh}_{hlf}")
            nc.gpsimd.partition_broadcast(brc, rcp[0:1, :])
            cols = slice(hlf * HALF, (hlf + 1) * HALF)
            if hh == 0:
                nc.vector.tensor_mul(
                    outt_sb[0:F, q, cols], ot[0:F, :], brc[0:F, :])
            else:
                nc.vector.tensor_mul(
                    outt_sb[F:2 * F, q, cols], ot[0:F, :], brc[F:2 * F, :])

        def y_proj(i, q0, q1, y_ps):
            """Y contribution of pairs [q0, q1) for s-tile i into y_ps."""
            for q in range(q0, q1):
                for hlf in range(2):
                    nc.tensor.matmul(
                        y_ps[:, hlf * HALF:(hlf + 1) * HALF],
                        lhsT=outt_sb[:, q, i * P:(i + 1) * P],
                        rhs=wp_sb[:, q, hlf * HALF:(hlf + 1) * HALF],
                        start=(q == q0), stop=(q == q1 - 1),
                        skip_group_check=True,
                    )

        # ---- prep all pairs, then software-pipelined attention ----
        for q in range(NP):
            prep(q)
        alloc_e(0)

        # gram(0) emitted alone; exps start as soon as tiles land
        for i in range(NT):
            for hh in range(2):
                gram_tile(0, i, hh)

        for q in range(NP):
            nxt = q + 1 if q + 1 < NP else None
            if nxt is not None:
                alloc_e(nxt)
            # interleave: gram(nxt) paced by ACT; out^T(q) + chains fill PE;
            # Y partial for pairs 0-3 emitted in windows q>=5
            chains = [(hh, hlf) for hh in range(2) for hlf in range(2)]
            for step in range(NT * 2):  # 16 gram steps per pair
                i, hh = step // 2, step % 2
                if nxt is not None:
                    gram_tile(nxt, i, hh)
                if step % 4 == 3:
                    c_hh, c_hlf = chains[step // 4]
                    ot_chain(q, c_hh, c_hlf)
            if q >= 5:
                # Y partial (pairs 0-3) for tiles assigned to this window
                if "ya" not in ya_ref:
                    ya_ref["ya"] = wtiles.tile([P, KO, S], BF16, tag="w",
                                               name="ya_sb")
                ya_sb = ya_ref["ya"]
                lo = (q - 5) * 3
                hi = min(lo + 3, NT)
                for i in range(lo, hi):
                    y_ps = ps_big.tile([P, D], F32, tag="big", name=f"ya_{i}")
                    y_proj(i, 0, NP // 2, y_ps)
                    nc.vector.tensor_add(ya_sb[:, i, :], y_ps, bp_sb)
            if q in e_store and q < NP - 1:
                del e_store[q]

        # tail: out^T(7) chains then Y_B + final add + DMA out
        for hh in range(2):
            for hlf in range(2):
                ot_chain(NP - 1, hh, hlf)
        del e_store[NP - 1]

        ya_sb = ya_ref["ya"]
        for i in range(NT):
            y_ps = ps_big.tile([P, D], F32, tag="big", name=f"yb_{i}")
            y_proj(i, NP // 2, NP, y_ps)
            y_sb = y_pool.tile([P, D], F32, tag="y", name=f"ys_{i}")
            nc.vector.tensor_add(y_sb, y_ps, ya_sb[:, i, :])
            nc.sync.dma_start(d_y[i * P:(i + 1) * P, :], y_sb)


_CACHE: dict = {}


def _get_program() -> bass.Bass:
    if "nc" not in _CACHE:
        _CACHE["nc"] = build_program()
    return _CACHE["nc"]


def _prep_inputs(sin, Wx, bx, Wp, bp):
    """Host-side sharding + layout prep. Returns per-core input maps."""
    bf16 = ml_dtypes.bfloat16
    wx_flat = np.ascontiguousarray(
        np.transpose(np.asarray(Wx, np.float32), (1, 0, 2)).reshape(D, H * F)
    ).astype(bf16)
    wp_b = np.ascontiguousarray(np.asarray(Wp, np.float32)).astype(bf16)
    bx32 = np.asarray(bx, np.float32)
    bxf = np.ascontiguousarray(bx32.reshape(1, H * F))
    bp32 = np.ascontiguousarray(np.asarray(bp, np.float32).reshape(1, D))
    # sel8[r][j][p] = 1 iff r == (p//64)*8 + j  (broadcasts invq rows j and
    # 8+j of a pair's [16,128] 1/|x| tile to partitions 0-63 / 64-127)
    sel8 = np.zeros((2 * NT, NT, P), np.float32)
    for j in range(NT):
        sel8[j, j, :F] = 1.0
        sel8[NT + j, j, F:] = 1.0
    sel8 = sel8.astype(bf16)
    ident = np.eye(P, dtype=np.float32).astype(bf16)

    sin32 = np.asarray(sin, np.float32)
    in_maps = []
    for b in range(B):
        sint = np.ascontiguousarray(sin32[b].T).astype(bf16)
        in_maps.append({
            "sint": sint, "wx": wx_flat, "wp": wp_b, "bxf": bxf,
            "bp": bp32, "sel8": sel8, "ident": ident,
        })
    return in_maps


def kernel(sin, mask, Wx, bx, Wp, bp, _run_kwargs=None):
    nc = _get_program()
    in_maps = _prep_inputs(sin, Wx, bx, Wp, bp)
    res = run_bass_kernel_spmd(nc, in_maps, core_ids=list(range(B)),
                               **(_run_kwargs or {}))
    out = np.stack([np.asarray(res.results[b]["y"], np.float32) for b in range(B)])
    if _run_kwargs:
        _CACHE["last_results"] = res
    return out


# revision 16
# speedup vs baseline: 1.2563x; 1.0053x over previous
"""Trainium2 Bass kernel for nn_MultiHeadAttention_91027536871977.

Cosine-similarity multi-head self-attention:
  x      = einsum("bsd,hdf->bhsf", sin, Wx) + bx          [B,H,S,F]
  scores = (x @ x^T) / (|x| |x|^T)                        [B,H,S,S]
  p      = softmax(scores, -1)
  out    = concat_heads(p @ x) @ Wp + bp                  [B,S,D]

Sharding: pure data-parallel over batch (B=8 -> 8 cores, one batch each,
all 16 heads + the output projection local to the core; no collectives).

v2 schedule (vs the 299us baseline):
  - x stored [t_p, tile, h, 65] with a ones column per head; the out^T
    matmuls use lhsT = [x_h | 1] (M=65) so PSUM row 64 accumulates
    rs = sum_t E[t,s] for free -> no ACT accum_out (saves ~40us of
    ACTIVATION_READ_ACCUMULATOR) and no selector-matmul rs chain.
  - 1/rs: DVE reciprocal of the PSUM rs row (bf16), GpSimd
    partition_broadcast to all 128 partitions, then mixed-partition-base
    DVE muls scale both heads' out^T halves (head1 writes parts 64-127
    directly from the base-0 PSUM tile).
  - |x|^2 (square + reduce) moved to GpSimd; DVE keeps only PSUM-touching
    work.
  - Y projection split: pairs 0-3 projected mid-loop into yA (f32 SBUF),
    pairs 4-7 + final add in the tail -> shorter ACT-idle tail.
  - Loop orders chosen so consecutive matmuls reuse the stationary
    operand where possible (fewer LDWEIGHTS stalls), and the PE stream is
    kept dense to hold the HAM clock gate at 2.4 GHz.
"""

import numpy as np
import ml_dtypes

import concourse.bass as bass
import concourse.bacc as bacc
import concourse.mybir as mybir
import concourse.tile as tile
from concourse.bass_utils import run_bass_kernel_spmd

B, S, D, H, F = 8, 1024, 1024, 16, 64
P = 128
NP = H // 2  # head pairs
KO = D // P  # k subtiles
NT = S // P  # s tiles
BF16 = mybir.dt.bfloat16
F32 = mybir.dt.float32
HALF = S // 2
FP = F + 1  # per-head x columns incl. the ones column


def build_program() -> bass.Bass:
    nc = bacc.Bacc("TRN2", target_bir_lowering=False, debug=False)

    d_sint = nc.dram_tensor("sint", [D, S], BF16, kind="ExternalInput")
    d_wx = nc.dram_tensor("wx", [D, H * F], BF16, kind="ExternalInput")
    d_wp = nc.dram_tensor("wp", [H * F, D], BF16, kind="ExternalInput")
    d_bxf = nc.dram_tensor("bxf", [1, H * F], F32, kind="ExternalInput")
    d_bp = nc.dram_tensor("bp", [1, D], F32, kind="ExternalInput")
    d_sel8 = nc.dram_tensor("sel8", [2 * NT, NT, P], BF16, kind="ExternalInput")
    d_ident = nc.dram_tensor("ident", [P, P], BF16, kind="ExternalInput")
    d_y = nc.dram_tensor("y", [S, D], F32, kind="ExternalOutput")

    with tile.TileContext(nc) as tc:
        _body(tc, d_sint, d_wx, d_wp, d_bxf, d_bp, d_sel8, d_ident, d_y)
    nc.compile()
    return nc


def _bcast_rows(dram_ap, parts=P):
    """DMA access pattern replicating a [1, N] DRAM row across `parts` partitions."""
    return bass.AP(
        tensor=dram_ap.tensor,
        offset=dram_ap.offset,
        ap=[[0, parts]] + list(dram_ap.ap[1:]),
    )


def _body(tc, d_sint, d_wx, d_wp, d_bxf, d_bp, d_sel8, d_ident, d_y):
    nc = tc.nc
    from contextlib import ExitStack

    with ExitStack() as ctx:
        singles = ctx.enter_context(tc.tile_pool(name="singles", bufs=1))
        wtiles = ctx.enter_context(tc.tile_pool(name="wtiles", bufs=2))
        e_pool = ctx.enter_context(tc.tile_pool(name="epool", bufs=4))
        b_pool = ctx.enter_context(tc.tile_pool(name="bpool", bufs=2))
        rc_pool = ctx.enter_context(tc.tile_pool(name="rcpool", bufs=2))
        y_pool = ctx.enter_context(tc.tile_pool(name="ypool", bufs=2))
        bc_pool = ctx.enter_context(tc.tile_pool(name="bcpool", bufs=1))

        # PSUM is bank-granular (8 x 2KB): big 2x2 banks + prep 2x1 bank
        # (bufs=1 per tag) + ot 2x1 bank = 8 banks
        ps_big = ctx.enter_context(tc.tile_pool(name="ps_big", bufs=2, space="PSUM"))
        ps_prep = ctx.enter_context(tc.tile_pool(name="ps_prep", bufs=1, space="PSUM"))
        ps_ot = ctx.enter_context(tc.tile_pool(name="ps_ot", bufs=2, space="PSUM"))

        # ---- load everything to SBUF ----
        # sint/wx are dead after the X projection; yA later rotates into
        # sint's buffer via the shared 2-buf tag.
        sint_sb = wtiles.tile([P, KO, S], BF16, tag="w", name="sint_sb")
        wx_sb = wtiles.tile([P, KO, H * F], BF16, tag="w", name="wx_sb")
        sint_r = d_sint.rearrange("(ko p) s -> p ko s", p=P)
        wx_r = d_wx.rearrange("(ko p) n -> p ko n", p=P)
        for ko in range(KO):
            nc.sync.dma_start(wx_sb[:, ko, :], wx_r[:, ko, :])
            nc.sync.dma_start(sint_sb[:, ko, :], sint_r[:, ko, :])
        wp_sb = singles.tile([P, KO, D], BF16)
        nc.sync.dma_start(wp_sb, d_wp.rearrange("(ko p) n -> p ko n", p=P))
        bxf_sb = bc_pool.tile([P, H * F], F32, tag="bc", name="bxf_sb")
        nc.gpsimd.dma_start(bxf_sb, _bcast_rows(d_bxf[:, :]))
        bp_sb = bc_pool.tile([P, D], F32, tag="bc", name="bp_sb")
        nc.gpsimd.dma_start(bp_sb, _bcast_rows(d_bp[:, :]))
        sel8_sb = singles.tile([2 * NT, NT, P], BF16)
        nc.sync.dma_start(sel8_sb, d_sel8[:, :, :])
        ident_sb = singles.tile([P, P], BF16)
        nc.sync.dma_start(ident_sb, d_ident[:, :])

        # persistent intermediates
        x_sb = singles.tile([P, NT, H, FP], BF16)   # x + ones col, [t_p, tile, h, f|1]
        xtn_sb = singles.tile([P, NP, S], BF16)     # normalized x^T [f2, pair, t]
        outt_sb = singles.tile([P, NP, S], BF16)    # attention out^T [f2, pair, s]
        n2s_sb = singles.tile([P, P], F32)          # |x|^2 [s_p, col h*8+i]
        nrcp_sb = singles.tile([P, P], F32)         # 1/|x|^2
        invs_sb = singles.tile([P, P], BF16)        # 1/|x| [s_p, col h*8+i]
        ya_ref = {}  # Y partial (pairs 0-3) + bias; allocated lazily

        # ones columns of x (written once; bias-add below fills cols 0:64)
        nc.vector.memset(x_sb[:, :, :, F:FP], 1.0)

        # ---- X = sin @ Wx + bx;  |x|^2 on GpSimd ----
        for i in range(NT):
            x_ps = ps_big.tile([P, S], F32, tag="big", name=f"x_{i}")
            for ko in range(KO):
                for hlf in range(2):
                    nc.tensor.matmul(
                        x_ps[:, hlf * HALF:(hlf + 1) * HALF],
                        lhsT=sint_sb[:, ko, i * P:(i + 1) * P],
                        rhs=wx_sb[:, ko, hlf * HALF:(hlf + 1) * HALF],
                        start=(ko == 0), stop=(ko == KO - 1),
                        skip_group_check=True,
                    )
            nc.vector.tensor_add(
                x_sb[:, i, :, 0:F],
                x_ps.rearrange("p (h f) -> p h f", f=F),
                bxf_sb.rearrange("p (h f) -> p h f", f=F),
            )
            xsq = b_pool.tile([P, H * F], BF16, tag="xsq", name=f"xsq_{i}")
            nc.gpsimd.tensor_mul(
                xsq.rearrange("p (h f) -> p h f", f=F),
                x_sb[:, i, :, 0:F], x_sb[:, i, :, 0:F],
            )
            nc.vector.reduce_sum(
                n2s_sb.rearrange("p (hh ii) -> p hh ii", ii=NT)[:, :, i],
                xsq.rearrange("p (hh f) -> p hh f", f=F),
                axis=mybir.AxisListType.X,
            )
        nc.vector.reciprocal(nrcp_sb, n2s_sb)
        nc.scalar.sqrt(invs_sb, nrcp_sb)

        def prep(q):
            """xtn for pair q: transpose x columns, scale by 1/|x| bcast."""
            invq_ps = ps_prep.tile([P, P], BF16, tag="xtt", name=f"invq_{q}")
            nc.tensor.transpose(
                invq_ps[0:2 * NT, :], invs_sb[:, q * 2 * NT:(q + 1) * 2 * NT],
                ident_sb)
            invq_sb = b_pool.tile([2 * NT, P], BF16, tag="rcpq", name=f"invqs_{q}")
            nc.vector.tensor_copy(invq_sb, invq_ps[0:2 * NT, :])
            nrm_sb = b_pool.tile([P, NT, P], BF16, tag="nrm", name=f"nrm_{q}")
            for j in range(NT):
                nrm_ps = ps_prep.tile([P, P], F32, tag="nrmp", name=f"nrmp_{q}_{j}")
                nc.tensor.matmul(
                    nrm_ps, lhsT=sel8_sb[:, j, :], rhs=invq_sb,
                    start=True, stop=True,
                )
                nc.vector.tensor_copy(nrm_sb[:, j, :], nrm_ps)
                xtt_ps = ps_prep.tile([P, P], BF16, tag="xtt", name=f"xtt_{q}_{j}")
                nc.tensor.transpose(
                    xtt_ps[0:F, :], x_sb[:, j, 2 * q, 0:F], ident_sb)
                nc.tensor.transpose(
                    xtt_ps[F:2 * F, :], x_sb[:, j, 2 * q + 1, 0:F], ident_sb)
                nc.vector.tensor_mul(
                    xtn_sb[:, q, j * P:(j + 1) * P], xtt_ps, nrm_sb[:, j, :])

        e_store = {}

        def gram_tile(q, i, hh):
            """Gram + exp for head hh of pair q at s-tile i."""
            g_ps = ps_big.tile([P, S], F32, tag="big", name=f"g_{q}_{hh}_{i}")
            frows = slice(hh * F, (hh + 1) * F)
            for hlf in range(2):
                nc.tensor.matmul(
                    g_ps[:, hlf * HALF:(hlf + 1) * HALF],
                    lhsT=xtn_sb[frows, q, i * P:(i + 1) * P],
                    rhs=xtn_sb[frows, q, hlf * HALF:(hlf + 1) * HALF],
                    start=True, stop=True,
                    skip_group_check=True,
                )
            nc.scalar.activation(
                e_store[q][hh][:, i, :], g_ps,
                mybir.ActivationFunctionType.Exp,
            )

        def alloc_e(q):
            e_store[q] = [
                e_pool.tile([P, NT, S], BF16, tag="e", name=f"e_{q}_{hh}")
                for hh in range(2)]

        def ot_chain(q, hh, hlf):
            """out^T accumulation for (pair q, head hh, s-half hlf) with the
            ones-column rs trick; returns nothing (writes outt_sb)."""
            ot = ps_ot.tile([FP, HALF], F32, tag="ot", name=f"ot_{q}_{hh}_{hlf}")
            for j in range(NT):
                nc.tensor.matmul(
                    ot,
                    lhsT=x_sb[:, j, 2 * q + hh, :],
                    rhs=e_store[q][hh][:, j, hlf * HALF:(hlf + 1) * HALF],
                    start=(j == 0), stop=(j == NT - 1),
                    skip_group_check=True,
                )
            rcp = rc_pool.tile([P, HALF], F32, tag="rcp", name=f"rcp_{q}_{hh}_{hlf}")
            # partition shift 64 -> 0: partition_broadcast reads its source
            # tile's partition 0.  approx_fast: 18 bits, single DVE pass.
            nc.vector.reciprocal_approx_fast(rcp[0:1, :], ot[F:FP, :])
            brc = rc_pool.tile([P, HALF], F32, tag="brc", name=f"brc_{q}_{hh}_{hlf}")
            nc.gpsimd.partition_broadcast(brc, rcp[0:1, :])
            cols = slice(hlf * HALF, (hlf + 1) * HALF)
            if hh == 0:
                nc.vector.tensor_mul(
                    outt_sb[0:F, q, cols], ot[0:F, :], brc[0:F, :])
            else:
                nc.vector.tensor_mul(
                    outt_sb[F:2 * F, q, cols], ot[0:F, :], brc[F:2 * F, :])

        def y_proj(i, q0, q1, y_ps):
            """Y contribution of pairs [q0, q1) for s-tile i into y_ps."""
            for q in range(q0, q1):
                for hlf in range(2):
                    nc.tensor.matmul(
                        y_ps[:, hlf * HALF:(hlf + 1) * HALF],
                        lhsT=outt_sb[:, q, i * P:(i + 1) * P],
                        rhs=wp_sb[:, q, hlf * HALF:(hlf + 1) * HALF],
                        start=(q == q0), stop=(q == q1 - 1),
                        skip_group_check=True,
                    )

        def filler():
            """Dummy transpose keeping the PE queue non-empty so the HAM
            clock gate stays at 8/8 during the ACT-paced loop."""
            fil = ps_prep.tile([P, P], BF16, tag="xtt", name="fil")
            nc.tensor.transpose(fil, ident_sb, ident_sb)

        # ---- prep(0,1); then gram(0) interleaved with prep(2..7) ----
        for q in range(2):
            prep(q)
        alloc_e(0)

        # gram(0) head-major so pair-0 chains can start after 8 exps;
        # prep(2..7) interleaved to keep the PE stream dense
        prep_left = list(range(2, NP))
        for hh in range(2):
            for i in range(NT):
                gram_tile(0, i, hh)
                if (hh * NT + i) % 3 == 2 and prep_left:
                    prep(prep_left.pop(0))
        while prep_left:
            prep(prep_left.pop(0))

        chains = [(hh, hlf) for hh in range(2) for hlf in range(2)]
        for q in range(NP - 1):
            nxt = q + 1
            alloc_e(nxt)
            # gram(nxt) paced by ACT; out^T(q) chains + yA + fillers keep
            # the PE busy in between
            for step in range(NT * 2):  # 16 gram steps per pair
                i, hh = step // 2, step % 2
                gram_tile(nxt, i, hh)
                filler()
                if step % 4 == 3:
                    c_hh, c_hlf = chains[step // 4]
                    ot_chain(q, c_hh, c_hlf)
                    filler()
            if 4 <= q <= 6:
                # Y partial (pairs 0-3), tiles spread over windows 4-6
                if "ya" not in ya_ref:
                    ya_ref["ya"] = wtiles.tile([P, KO, S], BF16, tag="w",
                                               name="ya_sb")
                ya_sb = ya_ref["ya"]
                lo = (q - 4) * 3
                hi = min(lo + 3, NT)
                for i in range(lo, hi):
                    y_ps = ps_big.tile([P, D], F32, tag="big", name=f"ya_{i}")
                    y_proj(i, 0, NP // 2, y_ps)
                    nc.vector.tensor_add(ya_sb[:, i, :], y_ps, bp_sb)
            if q in e_store and q < NP - 1:
                del e_store[q]

        # tail: out^T(7) chains then Y_B + final add + DMA out
        for hh in range(2):
            for hlf in range(2):
                ot_chain(NP - 1, hh, hlf)
        del e_store[NP - 1]

        ya_sb = ya_ref["ya"]
        for i in range(NT):
            y_ps = ps_big.tile([P, D], F32, tag="big", name=f"yb_{i}")
            y_proj(i, NP // 2, NP, y_ps)
            y_sb = y_pool.tile([P, D], F32, tag="y", name=f"ys_{i}")
            nc.vector.tensor_add(y_sb, y_ps, ya_sb[:, i, :])
            nc.sync.dma_start(d_y[i * P:(i + 1) * P, :], y_sb)


_CACHE: dict = {}


def _get_program() -> bass.Bass:
    if "nc" not in _CACHE:
        _CACHE["nc"] = build_program()
    return _CACHE["nc"]


def _prep_inputs(sin, Wx, bx, Wp, bp):
    """Host-side sharding + layout prep. Returns per-core input maps."""
    bf16 = ml_dtypes.bfloat16
    wx_flat = np.ascontiguousarray(
        np.transpose(np.asarray(Wx, np.float32), (1, 0, 2)).reshape(D, H * F)
    ).astype(bf16)
    wp_b = np.ascontiguousarray(np.asarray(Wp, np.float32)).astype(bf16)
    bx32 = np.asarray(bx, np.float32)
    bxf = np.ascontiguousarray(bx32.reshape(1, H * F))
    bp32 = np.ascontiguousarray(np.asarray(bp, np.float32).reshape(1, D))
    # sel8[r][j][p] = 1 iff r == (p//64)*8 + j  (broadcasts invq rows j and
    # 8+j of a pair's [16,128] 1/|x| tile to partitions 0-63 / 64-127)
    sel8 = np.zeros((2 * NT, NT, P), np.float32)
    for j in range(NT):
        sel8[j, j, :F] = 1.0
        sel8[NT + j, j, F:] = 1.0
    sel8 = sel8.astype(bf16)
    ident = np.eye(P, dtype=np.float32).astype(bf16)

    sin32 = np.asarray(sin, np.float32)
    in_maps = []
    for b in range(B):
        sint = np.ascontiguousarray(sin32[b].T).astype(bf16)
        in_maps.append({
            "sint": sint, "wx": wx_flat, "wp": wp_b, "bxf": bxf,
            "bp": bp32, "sel8": sel8, "ident": ident,
        })
    return in_maps


def kernel(sin, mask, Wx, bx, Wp, bp, _run_kwargs=None):
    nc = _get_program()
    in_maps = _prep_inputs(sin, Wx, bx, Wp, bp)
    res = run_bass_kernel_spmd(nc, in_maps, core_ids=list(range(B)),
                               **(_run_kwargs or {}))
    out = np.stack([np.asarray(res.results[b]["y"], np.float32) for b in range(B)])
    if _run_kwargs:
        _CACHE["last_results"] = res
    return out


# revision 22
# speedup vs baseline: 1.3750x; 1.0944x over previous
"""Trainium2 Bass kernel for nn_MultiHeadAttention_91027536871977.

Cosine-similarity multi-head self-attention:
  x      = einsum("bsd,hdf->bhsf", sin, Wx) + bx          [B,H,S,F]
  scores = (x @ x^T) / (|x| |x|^T)                        [B,H,S,S]
  p      = softmax(scores, -1)
  out    = concat_heads(p @ x) @ Wp + bp                  [B,S,D]

Sharding: pure data-parallel over batch (B=8 -> 8 cores, one batch each,
all 16 heads + the output projection local to the core; no collectives).

v2 schedule (vs the 299us baseline):
  - x stored [t_p, tile, h, 65] with a ones column per head; the out^T
    matmuls use lhsT = [x_h | 1] (M=65) so PSUM row 64 accumulates
    rs = sum_t E[t,s] for free -> no ACT accum_out (saves ~40us of
    ACTIVATION_READ_ACCUMULATOR) and no selector-matmul rs chain.
  - 1/rs: DVE reciprocal of the PSUM rs row (bf16), GpSimd
    partition_broadcast to all 128 partitions, then mixed-partition-base
    DVE muls scale both heads' out^T halves (head1 writes parts 64-127
    directly from the base-0 PSUM tile).
  - |x|^2 (square + reduce) moved to GpSimd; DVE keeps only PSUM-touching
    work.
  - Y projection split: pairs 0-3 projected mid-loop into yA (f32 SBUF),
    pairs 4-7 + final add in the tail -> shorter ACT-idle tail.
  - Loop orders chosen so consecutive matmuls reuse the stationary
    operand where possible (fewer LDWEIGHTS stalls), and the PE stream is
    kept dense to hold the HAM clock gate at 2.4 GHz.
"""

import numpy as np
import ml_dtypes

import concourse.bass as bass
import concourse.bacc as bacc
import concourse.mybir as mybir
import concourse.tile as tile
from concourse.bass_utils import run_bass_kernel_spmd

B, S, D, H, F = 8, 1024, 1024, 16, 64
P = 128
NP = H // 2  # head pairs
KO = D // P  # k subtiles
NT = S // P  # s tiles
BF16 = mybir.dt.bfloat16
F32 = mybir.dt.float32
HALF = S // 2
FP = F + 1  # per-head x columns incl. the ones column


def build_program() -> bass.Bass:
    nc = bacc.Bacc("TRN2", target_bir_lowering=False, debug=False)

    d_sint = nc.dram_tensor("sint", [D, S], BF16, kind="ExternalInput")
    d_wx = nc.dram_tensor("wx", [D, H * F], BF16, kind="ExternalInput")
    d_wp = nc.dram_tensor("wp", [H * F, D], BF16, kind="ExternalInput")
    d_bxf = nc.dram_tensor("bxf", [1, H * F], F32, kind="ExternalInput")
    d_bp = nc.dram_tensor("bp", [1, D], F32, kind="ExternalInput")
    d_sel8 = nc.dram_tensor("sel8", [2 * NT, NT, P], BF16, kind="ExternalInput")
    d_ident = nc.dram_tensor("ident", [P, P], BF16, kind="ExternalInput")
    d_y = nc.dram_tensor("y", [S, D], F32, kind="ExternalOutput")

    with tile.TileContext(nc) as tc:
        _body(tc, d_sint, d_wx, d_wp, d_bxf, d_bp, d_sel8, d_ident, d_y)
    nc.compile()
    return nc


def _bcast_rows(dram_ap, parts=P):
    """DMA access pattern replicating a [1, N] DRAM row across `parts` partitions."""
    return bass.AP(
        tensor=dram_ap.tensor,
        offset=dram_ap.offset,
        ap=[[0, parts]] + list(dram_ap.ap[1:]),
    )


def _body(tc, d_sint, d_wx, d_wp, d_bxf, d_bp, d_sel8, d_ident, d_y):
    nc = tc.nc
    from contextlib import ExitStack

    with ExitStack() as ctx:
        singles = ctx.enter_context(tc.tile_pool(name="singles", bufs=1))
        wtiles = ctx.enter_context(tc.tile_pool(name="wtiles", bufs=2))
        e_pool = ctx.enter_context(tc.tile_pool(name="epool", bufs=4))
        b_pool = ctx.enter_context(tc.tile_pool(name="bpool", bufs=2))
        rc_pool = ctx.enter_context(tc.tile_pool(name="rcpool", bufs=2))
        y_pool = ctx.enter_context(tc.tile_pool(name="ypool", bufs=2))
        bc_pool = ctx.enter_context(tc.tile_pool(name="bcpool", bufs=1))

        # PSUM is bank-granular (8 x 2KB): big 2x2 banks always; prep
        # (2x1 bank) lives only through the gram(0) phase, then its banks
        # are recycled for 4 ot buffers (deep out^T chain pipeline).
        ps_big = ctx.enter_context(tc.tile_pool(name="ps_big", bufs=2, space="PSUM"))
        pools = {}

        # ---- load everything to SBUF ----
        # sint/wx are dead after the X projection; yA later rotates into
        # sint's buffer via the shared 2-buf tag.
        sint_sb = wtiles.tile([P, KO, S], BF16, tag="w", name="sint_sb")
        wx_sb = wtiles.tile([P, KO, H * F], BF16, tag="w", name="wx_sb")
        sint_r = d_sint.rearrange("(ko p) s -> p ko s", p=P)
        wx_r = d_wx.rearrange("(ko p) n -> p ko n", p=P)
        for ko in range(KO):
            nc.sync.dma_start(wx_sb[:, ko, :], wx_r[:, ko, :])
            nc.sync.dma_start(sint_sb[:, ko, :], sint_r[:, ko, :])
        wp_sb = singles.tile([P, KO, D], BF16)
        nc.sync.dma_start(wp_sb, d_wp.rearrange("(ko p) n -> p ko n", p=P))
        bxf_sb = bc_pool.tile([P, H * F], F32, tag="bc", name="bxf_sb")
        nc.gpsimd.dma_start(bxf_sb, _bcast_rows(d_bxf[:, :]))
        bp_sb = bc_pool.tile([P, D], F32, tag="bc", name="bp_sb")
        nc.gpsimd.dma_start(bp_sb, _bcast_rows(d_bp[:, :]))
        sel8_sb = singles.tile([2 * NT, NT, P], BF16)
        nc.sync.dma_start(sel8_sb, d_sel8[:, :, :])
        ident_sb = singles.tile([P, P], BF16)
        nc.sync.dma_start(ident_sb, d_ident[:, :])

        # persistent intermediates
        x_sb = singles.tile([P, NT, H, FP], BF16)   # x + ones col, [t_p, tile, h, f|1]
        xtn_sb = singles.tile([P, NP, S], BF16)     # normalized x^T [f2, pair, t]
        outt_sb = singles.tile([P, NP, S], BF16)    # attention out^T [f2, pair, s]
        n2s_sb = singles.tile([P, P], F32)          # |x|^2 [s_p, col h*8+i]
        nrcp_sb = singles.tile([P, P], F32)         # 1/|x|^2
        invs_sb = singles.tile([P, P], BF16)        # 1/|x| [s_p, col h*8+i]
        ya_ref = {}  # Y partial (pairs 0-3) + bias; allocated lazily

        # ones columns of x (written once; bias-add below fills cols 0:64)
        nc.vector.memset(x_sb[:, :, :, F:FP], 1.0)

        # ---- X = sin @ Wx + bx;  |x|^2 on GpSimd ----
        for i in range(NT):
            x_ps = ps_big.tile([P, S], F32, tag="big", name=f"x_{i}")
            for ko in range(KO):
                for hlf in range(2):
                    nc.tensor.matmul(
                        x_ps[:, hlf * HALF:(hlf + 1) * HALF],
                        lhsT=sint_sb[:, ko, i * P:(i + 1) * P],
                        rhs=wx_sb[:, ko, hlf * HALF:(hlf + 1) * HALF],
                        start=(ko == 0), stop=(ko == KO - 1),
                        skip_group_check=True,
                    )
            nc.vector.tensor_add(
                x_sb[:, i, :, 0:F],
                x_ps.rearrange("p (h f) -> p h f", f=F),
                bxf_sb.rearrange("p (h f) -> p h f", f=F),
            )
            xsq = b_pool.tile([P, H * F], BF16, tag="xsq", name=f"xsq_{i}")
            nc.gpsimd.tensor_mul(
                xsq.rearrange("p (h f) -> p h f", f=F),
                x_sb[:, i, :, 0:F], x_sb[:, i, :, 0:F],
            )
            nc.vector.reduce_sum(
                n2s_sb.rearrange("p (hh ii) -> p hh ii", ii=NT)[:, :, i],
                xsq.rearrange("p (hh f) -> p hh f", f=F),
                axis=mybir.AxisListType.X,
            )
        nc.vector.reciprocal(nrcp_sb, n2s_sb)
        nc.scalar.sqrt(invs_sb, nrcp_sb)

        def prep(q):
            """xtn for pair q: transpose x columns, scale by 1/|x| bcast."""
            ps_prep = pools["prep"]
            invq_ps = ps_prep.tile([P, P], BF16, tag="xtt", name=f"invq_{q}")
            nc.tensor.transpose(
                invq_ps[0:2 * NT, :], invs_sb[:, q * 2 * NT:(q + 1) * 2 * NT],
                ident_sb)
            invq_sb = b_pool.tile([2 * NT, P], BF16, tag="rcpq", name=f"invqs_{q}")
            nc.vector.tensor_copy(invq_sb, invq_ps[0:2 * NT, :])
            nrm_sb = b_pool.tile([P, NT, P], BF16, tag="nrm", name=f"nrm_{q}")
            for j in range(NT):
                nrm_ps = ps_prep.tile([P, P], F32, tag="nrmp", name=f"nrmp_{q}_{j}")  # noqa
                nc.tensor.matmul(
                    nrm_ps, lhsT=sel8_sb[:, j, :], rhs=invq_sb,
                    start=True, stop=True,
                )
                nc.vector.tensor_copy(nrm_sb[:, j, :], nrm_ps)
                xtt_ps = ps_prep.tile([P, P], BF16, tag="xtt", name=f"xtt_{q}_{j}")  # noqa
                nc.tensor.transpose(
                    xtt_ps[0:F, :], x_sb[:, j, 2 * q, 0:F], ident_sb)
                nc.tensor.transpose(
                    xtt_ps[F:2 * F, :], x_sb[:, j, 2 * q + 1, 0:F], ident_sb)
                nc.vector.tensor_mul(
                    xtn_sb[:, q, j * P:(j + 1) * P], xtt_ps, nrm_sb[:, j, :])

        e_store = {}

        def gram_tile(q, i, hh):
            """Gram + exp for head hh of pair q at s-tile i."""
            g_ps = ps_big.tile([P, S], F32, tag="big", name=f"g_{q}_{hh}_{i}")
            frows = slice(hh * F, (hh + 1) * F)
            for hlf in range(2):
                nc.tensor.matmul(
                    g_ps[:, hlf * HALF:(hlf + 1) * HALF],
                    lhsT=xtn_sb[frows, q, i * P:(i + 1) * P],
                    rhs=xtn_sb[frows, q, hlf * HALF:(hlf + 1) * HALF],
                    start=True, stop=True,
                    skip_group_check=True,
                )
            nc.scalar.activation(
                e_store[q][hh][:, i, :], g_ps,
                mybir.ActivationFunctionType.Exp,
            )

        def alloc_e(q):
            e_store[q] = [
                e_pool.tile([P, NT, S], BF16, tag="e", name=f"e_{q}_{hh}")
                for hh in range(2)]

        def ot_chain(q, hh, hlf):
            """out^T accumulation for (pair q, head hh, s-half hlf) with the
            ones-column rs trick; returns nothing (writes outt_sb)."""
            ot = pools["ot"].tile([FP, HALF], F32, tag="ot", name=f"ot_{q}_{hh}_{hlf}")
            for j in range(NT):
                nc.tensor.matmul(
                    ot,
                    lhsT=x_sb[:, j, 2 * q + hh, :],
                    rhs=e_store[q][hh][:, j, hlf * HALF:(hlf + 1) * HALF],
                    start=(j == 0), stop=(j == NT - 1),
                    skip_group_check=True,
                )
            rcp = rc_pool.tile([P, HALF], F32, tag="rcp", name=f"rcp_{q}_{hh}_{hlf}")
            # partition shift 64 -> 0: partition_broadcast reads its source
            # tile's partition 0.  approx_fast: 18 bits, single DVE pass.
            nc.vector.reciprocal_approx_fast(rcp[0:1, :], ot[F:FP, :])
            brc = rc_pool.tile([P, HALF], F32, tag="brc", name=f"brc_{q}_{hh}_{hlf}")
            nc.gpsimd.partition_broadcast(brc, rcp[0:1, :])
            cols = slice(hlf * HALF, (hlf + 1) * HALF)
            if hh == 0:
                nc.vector.tensor_mul(
                    outt_sb[0:F, q, cols], ot[0:F, :], brc[0:F, :])
            else:
                nc.vector.tensor_mul(
                    outt_sb[F:2 * F, q, cols], ot[0:F, :], brc[F:2 * F, :])

        def y_proj(i, q0, q1, y_ps):
            """Y contribution of pairs [q0, q1) for s-tile i into y_ps."""
            for q in range(q0, q1):
                for hlf in range(2):
                    nc.tensor.matmul(
                        y_ps[:, hlf * HALF:(hlf + 1) * HALF],
                        lhsT=outt_sb[:, q, i * P:(i + 1) * P],
                        rhs=wp_sb[:, q, hlf * HALF:(hlf + 1) * HALF],
                        start=(q == q0), stop=(q == q1 - 1),
                        skip_group_check=True,
                    )

        # ---- prep(0,1); then gram(0) interleaved with prep(2..7) ----
        with tc.tile_pool(name="ps_prep", bufs=1, space="PSUM") as ps_prep_pool:
            pools["prep"] = ps_prep_pool
            for q in range(2):
                prep(q)
            alloc_e(0)

            # gram(0) head-major so pair-0 chains can start after 8 exps;
            # prep(2..7) interleaved to keep the PE stream dense
            prep_left = list(range(2, NP))
            for hh in range(2):
                for i in range(NT):
                    gram_tile(0, i, hh)
                    if (hh * NT + i) % 3 == 2 and prep_left:
                        prep(prep_left.pop(0))
            while prep_left:
                prep(prep_left.pop(0))

        ps_ot = ctx.enter_context(tc.tile_pool(name="ps_ot", bufs=4, space="PSUM"))
        pools["ot"] = ps_ot

        chains = [(hh, hlf) for hh in range(2) for hlf in range(2)]
        for q in range(NP - 1):
            nxt = q + 1
            alloc_e(nxt)
            # gram(nxt) paced by ACT; out^T(q) chains + yA keep the PE busy
            for step in range(NT * 2):  # 16 gram steps per pair
                i, hh = step // 2, step % 2
                gram_tile(nxt, i, hh)
                if step % 4 == 3:
                    c_hh, c_hlf = chains[step // 4]
                    ot_chain(q, c_hh, c_hlf)
            if 4 <= q <= 6:
                # Y partial (pairs 0-3), tiles spread over windows 4-6
                if "ya" not in ya_ref:
                    ya_ref["ya"] = wtiles.tile([P, KO, S], BF16, tag="w",
                                               name="ya_sb")
                ya_sb = ya_ref["ya"]
                lo = (q - 4) * 3
                hi = min(lo + 3, NT)
                for i in range(lo, hi):
                    y_ps = ps_big.tile([P, D], F32, tag="big", name=f"ya_{i}")
                    y_proj(i, 0, NP // 2, y_ps)
                    nc.vector.tensor_add(ya_sb[:, i, :], y_ps, bp_sb)
            if q in e_store and q < NP - 1:
                del e_store[q]

        # tail: out^T(7) chains then Y_B + final add + DMA out
        for hh in range(2):
            for hlf in range(2):
                ot_chain(NP - 1, hh, hlf)
        del e_store[NP - 1]

        ya_sb = ya_ref["ya"]
        for i in range(NT):
            y_ps = ps_big.tile([P, D], F32, tag="big", name=f"yb_{i}")
            y_proj(i, NP // 2, NP, y_ps)
            y_sb = y_pool.tile([P, D], F32, tag="y", name=f"ys_{i}")
            nc.vector.tensor_add(y_sb, y_ps, ya_sb[:, i, :])
            nc.sync.dma_start(d_y[i * P:(i + 1) * P, :], y_sb)


_CACHE: dict = {}


def _get_program() -> bass.Bass:
    if "nc" not in _CACHE:
        _CACHE["nc"] = build_program()
    return _CACHE["nc"]


def _prep_inputs(sin, Wx, bx, Wp, bp):
    """Host-side sharding + layout prep. Returns per-core input maps."""
    bf16 = ml_dtypes.bfloat16
    wx_flat = np.ascontiguousarray(
        np.transpose(np.asarray(Wx, np.float32), (1, 0, 2)).reshape(D, H * F)
    ).astype(bf16)
    wp_b = np.ascontiguousarray(np.asarray(Wp, np.float32)).astype(bf16)
    bx32 = np.asarray(bx, np.float32)
    bxf = np.ascontiguousarray(bx32.reshape(1, H * F))
    bp32 = np.ascontiguousarray(np.asarray(bp, np.float32).reshape(1, D))
    # sel8[r][j][p] = 1 iff r == (p//64)*8 + j  (broadcasts invq rows j and
    # 8+j of a pair's [16,128] 1/|x| tile to partitions 0-63 / 64-127)
    sel8 = np.zeros((2 * NT, NT, P), np.float32)
    for j in range(NT):
        sel8[j, j, :F] = 1.0
        sel8[NT + j, j, F:] = 1.0
    sel8 = sel8.astype(bf16)
    ident = np.eye(P, dtype=np.float32).astype(bf16)

    sin32 = np.asarray(sin, np.float32)
    in_maps = []
    for b in range(B):
        sint = np.ascontiguousarray(sin32[b].T).astype(bf16)
        in_maps.append({
            "sint": sint, "wx": wx_flat, "wp": wp_b, "bxf": bxf,
            "bp": bp32, "sel8": sel8, "ident": ident,
        })
    return in_maps


def kernel(sin, mask, Wx, bx, Wp, bp, _run_kwargs=None):
    nc = _get_program()
    in_maps = _prep_inputs(sin, Wx, bx, Wp, bp)
    res = run_bass_kernel_spmd(nc, in_maps, core_ids=list(range(B)),
                               **(_run_kwargs or {}))
    out = np.stack([np.asarray(res.results[b]["y"], np.float32) for b in range(B)])
    if _run_kwargs:
        _CACHE["last_results"] = res
    return out
